# revision 7
# baseline (speedup 1.0000x reference)
"""Trainium2 Bass kernel for nn_Attention_77146202570808.

Dual-stream (protein/molecule) multi-head attention block:
  q/k/v projections for both streams, 4 attention passes (pp, mm, pm, mp),
  a Linear over the *sequence* axis (P+M -> P / M), and output projections.

Sharding: data-parallel over batch. B=8 batches, 8 NeuronCores, one batch
per core. No collectives. Each core runs an identical program on its own
batch slice; weights are replicated to every core.

Layout strategy per core:
  - activations kept feature-major [D, S] for q/k (contraction over D_in),
    produced via PE-transpose of the [S, D] inputs.
  - v produced seq-major [S, D] directly (activations stationary), stored
    with a per-head ones column ([S, 12*(64+1)]) so the attention context
    matmul also produces the softmax denominator for free.
  - scores computed transposed sT[j, i] (lhsT = kT head slice, rhs = qT),
    exp on ScalarE straight out of PSUM (no max-subtraction: inputs are
    well-scaled so scores are tiny), ctx^T = v_aug^T @ probsT via PE with
    v_aug stationary, then PE-transpose back to seq-major and normalize.
  - fc over sequence: lhsT = cat tiles (seq-major), rhs = Wfc -> out_fcT
    feature-major; out projection: lhsT = out_fcT, rhs = Wout -> seq-major
    result, DMA'd out contiguously.
  - all matmul operands are float32r (full-speed fp32 mode, ~1.5e-4 rel).
"""

import numpy as np

import concourse.bass as bass
import concourse.mybir as mybir
import concourse.tile as tile
from concourse import bacc
from concourse import bass_utils
from concourse.masks import make_identity

F32 = mybir.dt.float32
F32R = mybir.dt.float32r
AF = mybir.ActivationFunctionType

B, P, M, D, H, DH = 8, 1024, 256, 768, 12, 64
S = P + M           # 1280
DT = D // 128       # 6 d-tiles
PT = P // 128       # 8
MT = M // 128       # 2
ST = S // 128       # 10
N_CORES = 8

_W_NAMES = ["Wq", "Wk", "Wv", "Wqm", "Wkm", "Wvm", "Wout", "Wout_mol"]
_B_NAMES = ["bq", "bk", "bv", "bqm", "bkm", "bvm", "bout", "bout_mol"]


def _chunks(total, size):
    out = []
    o = 0
    while o < total:
        out.append((o, min(size, total - o)))
        o += size
    return out


def _build():
    nc = bacc.Bacc("TRN2", target_bir_lowering=False, debug=False,
                   num_devices=N_CORES)

    io = {}
    io["hidden_states"] = nc.dram_tensor("hidden_states", [P, D], F32,
                                         kind="ExternalInput")
    io["mol"] = nc.dram_tensor("mol", [M, D], F32, kind="ExternalInput")
    for w in _W_NAMES:
        io[w] = nc.dram_tensor(w, [D, D], F32, kind="ExternalInput")
    for b in _B_NAMES:
        io[b] = nc.dram_tensor(b, [D], F32, kind="ExternalInput")
    io["Wfc"] = nc.dram_tensor("Wfc", [S, P], F32, kind="ExternalInput")
    io["bfc"] = nc.dram_tensor("bfc", [P], F32, kind="ExternalInput")
    io["Wfc_mol"] = nc.dram_tensor("Wfc_mol", [S, M], F32, kind="ExternalInput")
    io["bfc_mol"] = nc.dram_tensor("bfc_mol", [M], F32, kind="ExternalInput")
    io["out_prot"] = nc.dram_tensor("out_prot", [P, D], F32,
                                    kind="ExternalOutput")
    io["out_mol"] = nc.dram_tensor("out_mol", [M, D], F32,
                                   kind="ExternalOutput")
    # DRAM scratch for the concatenated attention contexts (seq-major).
    cat_prot = nc.dram_tensor("cat_prot", [S, D], F32R, kind="Internal")
    cat_mol = nc.dram_tensor("cat_mol", [S, D], F32R, kind="Internal")

    with tile.TileContext(nc) as tc:
        _kernel(tc, io, cat_prot, cat_mol)
    nc.compile()
    return nc


def _kernel(tc, io, cat_prot, cat_mol):
    nc = tc.nc
    ap = {k: v.ap() for k, v in io.items()}
    catp = cat_prot.ap().rearrange("(t p) d -> p t d", p=128)
    catm = cat_mol.ap().rearrange("(t p) d -> p t d", p=128)

    import contextlib
    ctx = contextlib.ExitStack()
    with ctx:
        const = ctx.enter_context(tc.tile_pool(name="const", bufs=1))
        psA = ctx.enter_context(tc.tile_pool(name="psA", bufs=2, space="PSUM"))
        psS = ctx.enter_context(tc.tile_pool(name="psS", bufs=2, space="PSUM"))
        psC = ctx.enter_context(tc.tile_pool(name="psC", bufs=2, space="PSUM"))

        # ---- constants -------------------------------------------------
        ident = const.tile([128, 128], F32)
        make_identity(nc, ident[:])

        def bcast(name, n):
            t = const.tile([128, n], F32, name=f"bc_{name}")
            src = ap[name].rearrange("(o n) -> o n", o=1).to_broadcast([128, n])
            nc.sync.dma_start(t[:], src)
            return t

        bv_b = bcast("bv", D)
        bvm_b = bcast("bvm", D)
        bfc_b = bcast("bfc", P)
        bfcm_b = bcast("bfc_mol", M)
        bout_b = bcast("bout", D)
        boutm_b = bcast("bout_mol", D)

        def ppart(name):
            t = const.tile([128, DT], F32, name=f"pp_{name}")
            nc.sync.dma_start(t[:], ap[name].rearrange("(mo p) -> p mo", p=128))
            return t

        bq_p, bk_p, bqm_p, bkm_p = ppart("bq"), ppart("bk"), ppart("bqm"), ppart("bkm")

        ones_c = const.tile([128, H], F32, name="ones_c")
        nc.vector.memset(ones_c[:], 1.0)

        # ---- activations that live through attention -------------------
        acts_cm = tc.tile_pool(name="acts", bufs=1)
        acts = acts_cm.__enter__()
        qT = acts.tile([128, DT, P], F32R)
        kT = acts.tile([128, DT, P], F32R)
        qmT = acts.tile([128, DT, M], F32R)
        kmT = acts.tile([128, DT, M], F32R)
        v_aug = acts.tile([128, PT, H * (DH + 1)], F32R)    # [128, 8, 780]
        vm_aug = acts.tile([128, MT, H * (DH + 1)], F32R)   # [128, 2, 780]

        # ---- phase 1: transpose inputs to feature-major ----------------
        with tc.tile_pool(name="proj", bufs=1) as proj:
            hsT = proj.tile([128, DT, P], F32R)
            molT = proj.tile([128, DT, M], F32R)
            with tc.tile_pool(name="ld", bufs=3) as ld:
                for (src, dstT, nt) in ((ap["hidden_states"], hsT, PT),
                                        (ap["mol"], molT, MT)):
                    for st in range(nt):
                        xs = ld.tile([128, D], F32, tag="xs")
                        nc.sync.dma_start(xs[:], src[st * 128:(st + 1) * 128, :])
                        for dt in range(DT):
                            pt = psA.tile([128, 512], F32, tag="psA")
                            nc.tensor.transpose(
                                pt[:, 0:128], xs[:, dt * 128:(dt + 1) * 128],
                                ident[:])
                            nc.vector.tensor_copy(
                                dstT[:, dt, st * 128:(st + 1) * 128],
                                pt[:, 0:128])

            # ---- phase 2: projections ---------------------------------
            with tc.tile_pool(name="wp", bufs=2) as wp:
                # feature-major outputs (q, k, qm, km)
                for (wname, bias_p, dstT, srcT, n_size) in (
                        ("Wq", bq_p, qT, hsT, P), ("Wk", bk_p, kT, hsT, P),
                        ("Wqm", bqm_p, qmT, molT, M),
                        ("Wkm", bkm_p, kmT, molT, M)):
                    w_sb = wp.tile([128, DT, D], F32R, tag="w", name=wname)
                    nc.gpsimd.dma_start(
                        w_sb[:], ap[wname].rearrange("(ko p) n -> p ko n", p=128))
                    for mo in range(DT):
                        for (n0, nsz) in _chunks(n_size, 512):
                            ps = psA.tile([128, 512], F32, tag="psA")
                            for ko in range(DT):
                                nc.tensor.matmul(
                                    ps[:, 0:nsz],
                                    w_sb[:, ko, mo * 128:(mo + 1) * 128],
                                    srcT[:, ko, n0:n0 + nsz],
                                    start=(ko == 0), stop=(ko == DT - 1))
                            nc.vector.tensor_scalar_add(
                                dstT[:, mo, n0:n0 + nsz], ps[:, 0:nsz],
                                bias_p[:, mo:mo + 1])

                # seq-major v with interleaved ones columns
                for (wname, bias_b, dst, srcT, seq_t) in (
                        ("Wv", bv_b, v_aug, hsT, PT),
                        ("Wvm", bvm_b, vm_aug, molT, MT)):
                    w_sb = wp.tile([128, DT, D], F32R, tag="w", name=wname)
                    nc.gpsimd.dma_start(
                        w_sb[:], ap[wname].rearrange("(ko p) n -> p ko n", p=128))
                    for st in range(seq_t):
                        for (n0, nsz) in _chunks(D, 512):
                            ps = psA.tile([128, 512], F32, tag="psA")
                            for ko in range(DT):
                                nc.tensor.matmul(
                                    ps[:, 0:nsz],
                                    srcT[:, ko, st * 128:(st + 1) * 128],
                                    w_sb[:, ko, n0:n0 + nsz],
                                    start=(ko == 0), stop=(ko == DT - 1))
                            h0, hn = n0 // DH, nsz // DH
                            dst_v = dst[:, st].rearrange(
                                "p (h x) -> p h x", x=DH + 1)[:, h0:h0 + hn, 0:DH]
                            nc.vector.tensor_add(
                                dst_v,
                                ps[:, 0:nsz].rearrange("p (h x) -> p h x", x=DH),
                                bias_b[:, n0:n0 + nsz].rearrange(
                                    "p (h x) -> p h x", x=DH))
                    for st in range(seq_t):
                        nc.vector.tensor_copy(
                            dst[:, st].rearrange(
                                "p (h x) -> p h x", x=DH + 1)[:, :, DH],
                            ones_c[:])

        # ---- phase 3: attention ---------------------------------------
        def attention(qsrc, SQ, ksrc, SK, vaug, cat_dst, st_base, tagsfx):
            JT = SK // 128
            CH = 512 if SQ >= 512 else SQ
            G = 2
            with tc.tile_pool(name=f"at_{tagsfx}", bufs=2) as at:
                for (i0, _) in _chunks(SQ, CH):
                    nst = CH // 128
                    stage = at.tile([128, nst, D], F32R, tag="stage")
                    for h in range(H):
                        dt, half = h // 2, (h % 2) * DH
                        ps_ct = psC.tile([128, CH], F32, tag="psC")
                        for jg in range(0, JT, G):
                            gn = min(G, JT - jg)
                            ps_s = psS.tile([128, G, CH], F32, tag="psS")
                            for g in range(gn):
                                jt = jg + g
                                nc.tensor.matmul(
                                    ps_s[:, g, :],
                                    ksrc[half:half + DH, dt,
                                         jt * 128:(jt + 1) * 128],
                                    qsrc[half:half + DH, dt, i0:i0 + CH],
                                    start=True, stop=True)
                            probs = at.tile([128, G, CH], F32R, tag="probs")
                            nc.scalar.activation(
                                probs[:, 0:gn, :], ps_s[:, 0:gn, :], AF.Exp,
                                scale=0.125)
                            for g in range(gn):
                                jt = jg + g
                                nc.tensor.matmul(
                                    ps_ct[0:DH + 1, :],
                                    vaug[:, jt, h * (DH + 1):(h + 1) * (DH + 1)],
                                    probs[:, g, :],
                                    start=(jt == 0), stop=(jt == JT - 1))
                        ctx_sb = at.tile([DH + 1, CH], F32, tag="ctx")
                        nc.vector.tensor_copy(ctx_sb[:], ps_ct[0:DH + 1, :])
                        for ii in range(nst):
                            ps_t = psA.tile([128, 512], F32, tag="psA")
                            nc.tensor.transpose(
                                ps_t[:, 0:DH + 1],
                                ctx_sb[:, ii * 128:(ii + 1) * 128],
                                ident[0:DH + 1, 0:DH + 1])
                            rec = at.tile([128, 1], F32, tag="rec", bufs=4)
                            nc.vector.reciprocal(rec[:], ps_t[:, DH:DH + 1])
                            nc.vector.tensor_scalar_mul(
                                stage[:, ii, h * DH:(h + 1) * DH],
                                ps_t[:, 0:DH], rec[:])
                    st = st_base + (i0 // 128)
                    nc.sync.dma_start(cat_dst[:, st:st + nst, :], stage[:])

        attention(qT, P, kT, P, v_aug, catp, 0, "pp")        # prot->prot
        attention(qmT, M, kT, P, v_aug, catp, PT, "mp")      # mol->prot kv
        attention(qmT, M, kmT, M, vm_aug, catm, 0, "mm")     # mol->mol
        attention(qT, P, kmT, M, vm_aug, catm, MT, "pm")     # prot->mol kv
        acts_cm.__exit__(None, None, None)

        # ---- phase 4: fc over the sequence axis -----------------------
        acts2 = ctx.enter_context(tc.tile_pool(name="acts2", bufs=1))
        ofcT = acts2.tile([128, DT, P], F32R)
        ofcmT = acts2.tile([128, DT, M], F32R)
        with tc.tile_pool(name="fc", bufs=1) as fc:
            for (cat_src, wname, bias_bc, dstT, NP) in (
                    (catp, "Wfc", bfc_b, ofcT, P),
                    (catm, "Wfc_mol", bfcm_b, ofcmT, M)):
                wfc_sb = fc.tile([128, ST, NP], F32R, name=f"sb_{wname}")
                nc.gpsimd.dma_start(
                    wfc_sb[:], ap[wname].rearrange("(ko p) n -> p ko n", p=128))
                cat_sb = fc.tile([128, ST, D], F32R, name=f"cat_{wname}")
                nc.sync.dma_start(cat_sb[:], cat_src[:])
                for mo in range(DT):
                    for (n0, nsz) in _chunks(NP, 512):
                        ps = psA.tile([128, 512], F32, tag="psA")
                        for st in range(ST):
                            nc.tensor.matmul(
                                ps[:, 0:nsz],
                                cat_sb[:, st, mo * 128:(mo + 1) * 128],
                                wfc_sb[:, st, n0:n0 + nsz],
                                start=(st == 0), stop=(st == ST - 1))
                        nc.vector.tensor_add(
                            dstT[:, mo, n0:n0 + nsz], ps[:, 0:nsz],
                            bias_bc[:, n0:n0 + nsz])

        # ---- phase 5: output projections ------------------------------
        with tc.tile_pool(name="op", bufs=1) as op:
            with tc.tile_pool(name="ost", bufs=3) as ost:
                for (srcT, wname, bias_bc, out_dram, n_tiles) in (
                        (ofcT, "Wout", bout_b, ap["out_prot"], PT),
                        (ofcmT, "Wout_mol", boutm_b, ap["out_mol"], MT)):
                    wo_sb = op.tile([128, DT, D], F32R, name=f"sb_{wname}")
                    nc.gpsimd.dma_start(
                        wo_sb[:], ap[wname].rearrange("(ko p) n -> p ko n", p=128))
                    for mo in range(n_tiles):
                        o_sb = ost.tile([128, D], F32, tag="osb")
                        for (n0, nsz) in _chunks(D, 512):
                            ps = psA.tile([128, 512], F32, tag="psA")
                            for kt in range(DT):
                                nc.tensor.matmul(
                                    ps[:, 0:nsz],
                                    srcT[:, kt, mo * 128:(mo + 1) * 128],
                                    wo_sb[:, kt, n0:n0 + nsz],
                                    start=(kt == 0), stop=(kt == DT - 1))
                            nc.vector.tensor_add(
                                o_sb[:, n0:n0 + nsz], ps[:, 0:nsz],
                                bias_bc[:, n0:n0 + nsz])
                        nc.sync.dma_start(
                            out_dram[mo * 128:(mo + 1) * 128, :], o_sb[:])


_NC_CACHE = None


def _get_program():
    global _NC_CACHE
    if _NC_CACHE is None:
        _NC_CACHE = _build()
    return _NC_CACHE


def kernel(**inputs):
    nc = _get_program()
    per_core_names = (["hidden_states", "mol"] + _W_NAMES + _B_NAMES
                      + ["Wfc", "bfc", "Wfc_mol", "bfc_mol"])
    in_maps = []
    for c in range(N_CORES):
        m = {}
        for name in per_core_names:
            arr = np.ascontiguousarray(np.asarray(inputs[name], dtype=np.float32))
            if name in ("hidden_states", "mol"):
                arr = arr[c]
            m[name] = arr
        in_maps.append(m)

    res = bass_utils.run_bass_kernel_spmd(nc, in_maps,
                                          core_ids=list(range(N_CORES)))
    global LAST_RESULTS
    LAST_RESULTS = res
    out_prot = np.stack([res.results[c]["out_prot"] for c in range(N_CORES)])
    out_mol = np.stack([res.results[c]["out_mol"] for c in range(N_CORES)])
    return out_prot, out_mol


LAST_RESULTS = None


# revision 8
# speedup vs baseline: 1.1500x; 1.1500x over previous
"""Trainium2 Bass kernel for nn_Attention_77146202570808.

Dual-stream (protein/molecule) multi-head attention block:
  q/k/v projections for both streams, 4 attention passes (pp, mm, pm, mp),
  a Linear over the *sequence* axis (P+M -> P / M), and output projections.

Sharding: data-parallel over batch. B=8 batches, 8 NeuronCores, one batch
per core. No collectives. Each core runs an identical program on its own
batch slice; weights are replicated to every core.

Layout strategy per core:
  - activations kept feature-major [D, S] for q/k (contraction over D_in),
    produced via PE-transpose of the [S, D] inputs.
  - v produced seq-major [S, D] directly (activations stationary), stored
    with a per-head ones column ([S, 12*(64+1)]) so the attention context
    matmul also produces the softmax denominator for free.
  - scores computed transposed sT[j, i] (lhsT = kT head slice, rhs = qT),
    exp on ScalarE straight out of PSUM (no max-subtraction: inputs are
    well-scaled so scores are tiny), ctx^T = v_aug^T @ probsT via PE with
    v_aug stationary, then PE-transpose back to seq-major and normalize.
  - fc over sequence: lhsT = cat tiles (seq-major), rhs = Wfc -> out_fcT
    feature-major; out projection: lhsT = out_fcT, rhs = Wout -> seq-major
    result, DMA'd out contiguously.
  - all matmul operands are float32r (full-speed fp32 mode, ~1.5e-4 rel).
"""

import numpy as np

import concourse.bass as bass
import concourse.mybir as mybir
import concourse.tile as tile
from concourse import bacc
from concourse import bass_utils
from concourse.masks import make_identity

F32 = mybir.dt.float32
F32R = mybir.dt.float32r
AF = mybir.ActivationFunctionType

B, P, M, D, H, DH = 8, 1024, 256, 768, 12, 64
S = P + M           # 1280
DT = D // 128       # 6 d-tiles
PT = P // 128       # 8
MT = M // 128       # 2
ST = S // 128       # 10
N_CORES = 8

_W_NAMES = ["Wq", "Wk", "Wv", "Wqm", "Wkm", "Wvm", "Wout", "Wout_mol"]
_B_NAMES = ["bq", "bk", "bv", "bqm", "bkm", "bvm", "bout", "bout_mol"]


def _chunks(total, size):
    out = []
    o = 0
    while o < total:
        out.append((o, min(size, total - o)))
        o += size
    return out


def _build():
    nc = bacc.Bacc("TRN2", target_bir_lowering=False, debug=False,
                   num_devices=N_CORES)

    io = {}
    io["hidden_states"] = nc.dram_tensor("hidden_states", [P, D], F32,
                                         kind="ExternalInput")
    io["mol"] = nc.dram_tensor("mol", [M, D], F32, kind="ExternalInput")
    for w in _W_NAMES:
        io[w] = nc.dram_tensor(w, [D, D], F32, kind="ExternalInput")
    for b in _B_NAMES:
        io[b] = nc.dram_tensor(b, [D], F32, kind="ExternalInput")
    io["Wfc"] = nc.dram_tensor("Wfc", [S, P], F32, kind="ExternalInput")
    io["bfc"] = nc.dram_tensor("bfc", [P], F32, kind="ExternalInput")
    io["Wfc_mol"] = nc.dram_tensor("Wfc_mol", [S, M], F32, kind="ExternalInput")
    io["bfc_mol"] = nc.dram_tensor("bfc_mol", [M], F32, kind="ExternalInput")
    io["out_prot"] = nc.dram_tensor("out_prot", [P, D], F32,
                                    kind="ExternalOutput")
    io["out_mol"] = nc.dram_tensor("out_mol", [M, D], F32,
                                   kind="ExternalOutput")
    # DRAM scratch for the concatenated attention contexts (seq-major).
    cat_prot = nc.dram_tensor("cat_prot", [S, D], F32R, kind="Internal")
    cat_mol = nc.dram_tensor("cat_mol", [S, D], F32R, kind="Internal")

    with tile.TileContext(nc) as tc:
        _kernel(tc, io, cat_prot, cat_mol)
    nc.compile()
    return nc


def _kernel(tc, io, cat_prot, cat_mol):
    nc = tc.nc
    ap = {k: v.ap() for k, v in io.items()}
    catp = cat_prot.ap().rearrange("(t p) d -> p t d", p=128)
    catm = cat_mol.ap().rearrange("(t p) d -> p t d", p=128)

    import contextlib
    ctx = contextlib.ExitStack()
    with ctx:
        const = ctx.enter_context(tc.tile_pool(name="const", bufs=1))
        psA = ctx.enter_context(tc.tile_pool(name="psA", bufs=2, space="PSUM"))
        psS = ctx.enter_context(tc.tile_pool(name="psS", bufs=2, space="PSUM"))
        psC = ctx.enter_context(tc.tile_pool(name="psC", bufs=2, space="PSUM"))

        # ---- constants -------------------------------------------------
        ident = const.tile([128, 128], F32)
        make_identity(nc, ident[:])

        def bcast(name, n):
            t = const.tile([128, n], F32, name=f"bc_{name}")
            src = ap[name].rearrange("(o n) -> o n", o=1).to_broadcast([128, n])
            nc.sync.dma_start(t[:], src)
            return t

        bv_b = bcast("bv", D)
        bvm_b = bcast("bvm", D)
        bfc_b = bcast("bfc", P)
        bfcm_b = bcast("bfc_mol", M)
        bout_b = bcast("bout", D)
        boutm_b = bcast("bout_mol", D)

        def ppart(name):
            t = const.tile([128, DT], F32, name=f"pp_{name}")
            nc.sync.dma_start(t[:], ap[name].rearrange("(mo p) -> p mo", p=128))
            return t

        bq_p, bk_p, bqm_p, bkm_p = ppart("bq"), ppart("bk"), ppart("bqm"), ppart("bkm")

        ones_c = const.tile([128, H], F32, name="ones_c")
        nc.vector.memset(ones_c[:], 1.0)

        # ---- activations that live through attention -------------------
        acts_cm = tc.tile_pool(name="acts", bufs=1)
        acts = acts_cm.__enter__()
        qT = acts.tile([128, DT, P], F32R)
        kT = acts.tile([128, DT, P], F32R)
        qmT = acts.tile([128, DT, M], F32R)
        kmT = acts.tile([128, DT, M], F32R)
        v_aug = acts.tile([128, PT, H * (DH + 1)], F32R)    # [128, 8, 780]
        vm_aug = acts.tile([128, MT, H * (DH + 1)], F32R)   # [128, 2, 780]

        # ---- phase 1: transpose inputs to feature-major ----------------
        with tc.tile_pool(name="proj", bufs=1) as proj:
            hsT = proj.tile([128, DT, P], F32R)
            molT = proj.tile([128, DT, M], F32R)
            with tc.tile_pool(name="ld", bufs=3) as ld, \
                    nc.named_scope("transpose_in"):
                for (src, dstT, nt) in ((ap["hidden_states"], hsT, PT),
                                        (ap["mol"], molT, MT)):
                    for st in range(nt):
                        xs = ld.tile([128, D], F32, tag="xs")
                        nc.sync.dma_start(xs[:], src[st * 128:(st + 1) * 128, :])
                        for dt in range(DT):
                            pt = psA.tile([128, 512], F32, tag="psA")
                            nc.tensor.transpose(
                                pt[:, 0:128], xs[:, dt * 128:(dt + 1) * 128],
                                ident[:])
                            nc.vector.tensor_copy(
                                dstT[:, dt, st * 128:(st + 1) * 128],
                                pt[:, 0:128])

            # ---- phase 2: projections ---------------------------------
            with tc.tile_pool(name="wp", bufs=2) as wp, \
                    nc.named_scope("proj"):
                # feature-major outputs (q, k, qm, km)
                for (wname, bias_p, dstT, srcT, n_size) in (
                        ("Wq", bq_p, qT, hsT, P), ("Wk", bk_p, kT, hsT, P),
                        ("Wqm", bqm_p, qmT, molT, M),
                        ("Wkm", bkm_p, kmT, molT, M)):
                    w_sb = wp.tile([128, DT, D], F32R, tag="w", name=wname)
                    nc.gpsimd.dma_start(
                        w_sb[:], ap[wname].rearrange("(ko p) n -> p ko n", p=128))
                    for mo in range(DT):
                        for (n0, nsz) in _chunks(n_size, 512):
                            ps = psA.tile([128, 512], F32, tag="psA")
                            for ko in range(DT):
                                nc.tensor.matmul(
                                    ps[:, 0:nsz],
                                    w_sb[:, ko, mo * 128:(mo + 1) * 128],
                                    srcT[:, ko, n0:n0 + nsz],
                                    start=(ko == 0), stop=(ko == DT - 1))
                            nc.vector.tensor_scalar_add(
                                dstT[:, mo, n0:n0 + nsz], ps[:, 0:nsz],
                                bias_p[:, mo:mo + 1])

                # seq-major v with interleaved ones columns
                for (wname, bias_b, dst, srcT, seq_t) in (
                        ("Wv", bv_b, v_aug, hsT, PT),
                        ("Wvm", bvm_b, vm_aug, molT, MT)):
                    w_sb = wp.tile([128, DT, D], F32R, tag="w", name=wname)
                    nc.gpsimd.dma_start(
                        w_sb[:], ap[wname].rearrange("(ko p) n -> p ko n", p=128))
                    for st in range(seq_t):
                        for (n0, nsz) in _chunks(D, 512):
                            ps = psA.tile([128, 512], F32, tag="psA")
                            for ko in range(DT):
                                nc.tensor.matmul(
                                    ps[:, 0:nsz],
                                    srcT[:, ko, st * 128:(st + 1) * 128],
                                    w_sb[:, ko, n0:n0 + nsz],
                                    start=(ko == 0), stop=(ko == DT - 1))
                            h0, hn = n0 // DH, nsz // DH
                            dst_v = dst[:, st].rearrange(
                                "p (h x) -> p h x", x=DH + 1)[:, h0:h0 + hn, 0:DH]
                            nc.vector.tensor_add(
                                dst_v,
                                ps[:, 0:nsz].rearrange("p (h x) -> p h x", x=DH),
                                bias_b[:, n0:n0 + nsz].rearrange(
                                    "p (h x) -> p h x", x=DH))
                    for st in range(seq_t):
                        nc.vector.tensor_copy(
                            dst[:, st].rearrange(
                                "p (h x) -> p h x", x=DH + 1)[:, :, DH],
                            ones_c[:])

        # ---- phase 3: attention ---------------------------------------
        def attention(qsrc, SQ, ksrc, SK, vaug, cat_dst, st_base, tagsfx):
            JT = SK // 128
            CH = 512 if SQ >= 512 else SQ
            G = 2
            with tc.tile_pool(name=f"at_{tagsfx}", bufs=2) as at, \
                    nc.named_scope(f"att_{tagsfx}"):
                for (i0, _) in _chunks(SQ, CH):
                    nst = CH // 128
                    stage = at.tile([128, nst, D], F32R, tag="stage")
                    for h in range(H):
                        dt, half = h // 2, (h % 2) * DH
                        ps_ct = psC.tile([128, CH], F32, tag="psC")
                        for jg in range(0, JT, G):
                            gn = min(G, JT - jg)
                            ps_s = psS.tile([128, G, CH], F32, tag="psS")
                            for g in range(gn):
                                jt = jg + g
                                nc.tensor.matmul(
                                    ps_s[:, g, :],
                                    ksrc[half:half + DH, dt,
                                         jt * 128:(jt + 1) * 128],
                                    qsrc[half:half + DH, dt, i0:i0 + CH],
                                    start=True, stop=True)
                            probs = at.tile([128, G, CH], F32R, tag="probs")
                            nc.scalar.activation(
                                probs[:, 0:gn, :], ps_s[:, 0:gn, :], AF.Exp,
                                scale=0.125)
                            for g in range(gn):
                                jt = jg + g
                                nc.tensor.matmul(
                                    ps_ct[0:DH + 1, :],
                                    vaug[:, jt, h * (DH + 1):(h + 1) * (DH + 1)],
                                    probs[:, g, :],
                                    start=(jt == 0), stop=(jt == JT - 1))
                        ctx_sb = at.tile([DH + 1, CH], F32, tag="ctx")
                        nc.vector.tensor_copy(ctx_sb[:], ps_ct[0:DH + 1, :])
                        for ii in range(nst):
                            ps_t = psA.tile([128, 512], F32, tag="psA")
                            nc.tensor.transpose(
                                ps_t[:, 0:DH + 1],
                                ctx_sb[:, ii * 128:(ii + 1) * 128],
                                ident[0:DH + 1, 0:DH + 1])
                            rec = at.tile([128, 1], F32, tag="rec", bufs=4)
                            nc.vector.reciprocal(rec[:], ps_t[:, DH:DH + 1])
                            nc.vector.tensor_scalar_mul(
                                stage[:, ii, h * DH:(h + 1) * DH],
                                ps_t[:, 0:DH], rec[:])
                    st = st_base + (i0 // 128)
                    nc.sync.dma_start(cat_dst[:, st:st + nst, :], stage[:])

        attention(qT, P, kT, P, v_aug, catp, 0, "pp")        # prot->prot
        attention(qmT, M, kT, P, v_aug, catp, PT, "mp")      # mol->prot kv
        attention(qmT, M, kmT, M, vm_aug, catm, 0, "mm")     # mol->mol
        attention(qT, P, kmT, M, vm_aug, catm, MT, "pm")     # prot->mol kv
        acts_cm.__exit__(None, None, None)

        # ---- phase 4: fc over the sequence axis -----------------------
        acts2 = ctx.enter_context(tc.tile_pool(name="acts2", bufs=1))
        ofcT = acts2.tile([128, DT, P], F32R)
        ofcmT = acts2.tile([128, DT, M], F32R)
        with tc.tile_pool(name="fc", bufs=1) as fc, nc.named_scope("fc"):
            for (cat_src, wname, bias_bc, dstT, NP) in (
                    (catp, "Wfc", bfc_b, ofcT, P),
                    (catm, "Wfc_mol", bfcm_b, ofcmT, M)):
                wfc_sb = fc.tile([128, ST, NP], F32R, name=f"sb_{wname}")
                nc.gpsimd.dma_start(
                    wfc_sb[:], ap[wname].rearrange("(ko p) n -> p ko n", p=128))
                cat_sb = fc.tile([128, ST, D], F32R, name=f"cat_{wname}")
                nc.sync.dma_start(cat_sb[:], cat_src[:])
                for mo in range(DT):
                    for (n0, nsz) in _chunks(NP, 512):
                        ps = psA.tile([128, 512], F32, tag="psA")
                        for st in range(ST):
                            nc.tensor.matmul(
                                ps[:, 0:nsz],
                                cat_sb[:, st, mo * 128:(mo + 1) * 128],
                                wfc_sb[:, st, n0:n0 + nsz],
                                start=(st == 0), stop=(st == ST - 1))
                        nc.vector.tensor_add(
                            dstT[:, mo, n0:n0 + nsz], ps[:, 0:nsz],
                            bias_bc[:, n0:n0 + nsz])

        # ---- phase 5: output projections ------------------------------
        with tc.tile_pool(name="op", bufs=1) as op, nc.named_scope("outproj"):
            with tc.tile_pool(name="ost", bufs=3) as ost:
                for (srcT, wname, bias_bc, out_dram, n_tiles) in (
                        (ofcT, "Wout", bout_b, ap["out_prot"], PT),
                        (ofcmT, "Wout_mol", boutm_b, ap["out_mol"], MT)):
                    wo_sb = op.tile([128, DT, D], F32R, name=f"sb_{wname}")
                    nc.gpsimd.dma_start(
                        wo_sb[:], ap[wname].rearrange("(ko p) n -> p ko n", p=128))
                    for mo in range(n_tiles):
                        o_sb = ost.tile([128, D], F32, tag="osb")
                        for (n0, nsz) in _chunks(D, 512):
                            ps = psA.tile([128, 512], F32, tag="psA")
                            for kt in range(DT):
                                nc.tensor.matmul(
                                    ps[:, 0:nsz],
                                    srcT[:, kt, mo * 128:(mo + 1) * 128],
                                    wo_sb[:, kt, n0:n0 + nsz],
                                    start=(kt == 0), stop=(kt == DT - 1))
                            nc.vector.tensor_add(
                                o_sb[:, n0:n0 + nsz], ps[:, 0:nsz],
                                bias_bc[:, n0:n0 + nsz])
                        nc.sync.dma_start(
                            out_dram[mo * 128:(mo + 1) * 128, :], o_sb[:])


_NC_CACHE = None


def _get_program():
    global _NC_CACHE
    if _NC_CACHE is None:
        _NC_CACHE = _build()
    return _NC_CACHE


def kernel(**inputs):
    nc = _get_program()
    per_core_names = (["hidden_states", "mol"] + _W_NAMES + _B_NAMES
                      + ["Wfc", "bfc", "Wfc_mol", "bfc_mol"])
    in_maps = []
    for c in range(N_CORES):
        m = {}
        for name in per_core_names:
            arr = np.ascontiguousarray(np.asarray(inputs[name], dtype=np.float32))
            if name in ("hidden_states", "mol"):
                arr = arr[c]
            m[name] = arr
        in_maps.append(m)

    res = bass_utils.run_bass_kernel_spmd(nc, in_maps,
                                          core_ids=list(range(N_CORES)))
    global LAST_RESULTS
    LAST_RESULTS = res
    out_prot = np.stack([res.results[c]["out_prot"] for c in range(N_CORES)])
    out_mol = np.stack([res.results[c]["out_mol"] for c in range(N_CORES)])
    return out_prot, out_mol


LAST_RESULTS = None


# revision 11
# speedup vs baseline: 1.1610x; 1.0095x over previous
"""Trainium2 Bass kernel for nn_Attention_77146202570808.

Dual-stream (protein/molecule) multi-head attention block:
  q/k/v projections for both streams, 4 attention passes (pp, mm, pm, mp),
  a Linear over the *sequence* axis (P+M -> P / M), and output projections.

Sharding: data-parallel over batch. B=8 batches, 8 NeuronCores, one batch
per core. No collectives; weights replicated to every core.

Layout strategy per core:
  - activations kept feature-major [D, S] for q/k (contraction over D_in),
    produced via PE-transpose of the [S, D] inputs.
  - v produced seq-major [S, D] directly (activations stationary), stored
    with a per-head ones column ([S, 12*(64+1)]) so the attention context
    matmul also produces the softmax denominator for free.
  - scores computed transposed sT[j, i] (lhsT = kT head slice, rhs = qT);
    heads processed in pairs on opposite PE row halves so their K=64
    matmuls run concurrently (row-group concurrency). exp on ScalarE
    straight out of PSUM (no max-subtraction; scores are small).
    ctx^T = v_aug^T @ probsT with v_aug stationary (M=65 incl. the ones
    row), then PE-transpose back to seq-major and normalize.
  - matmuls of the same shape are emitted in streaks: alternating PE
    configurations (score<->ctx) measured ~400ns/MM vs ~230ns in streaks.
  - fc over sequence: lhsT = cat tiles (seq-major), rhs = Wfc -> out_fcT
    feature-major; out projection: lhsT = out_fcT, rhs = Wout -> seq-major
    result, DMA'd out contiguously. fc for the protein stream is emitted
    between the mp and mm attention streams so its PE work overlaps the
    ACT-heavy attention tail.
  - all matmul operands are float32r (full-speed fp32 mode, ~3e-4 rel).
"""

import contextlib

import numpy as np

import concourse.bass as bass
import concourse.mybir as mybir
import concourse.tile as tile
from concourse import bacc
from concourse import bass_utils
from concourse.masks import make_identity

F32 = mybir.dt.float32
F32R = mybir.dt.float32r
AF = mybir.ActivationFunctionType

B, P, M, D, H, DH = 8, 1024, 256, 768, 12, 64
S = P + M           # 1280
DT = D // 128       # 6 d-tiles
PT = P // 128       # 8
MT = M // 128       # 2
ST = S // 128       # 10
N_CORES = 8

_W_NAMES = ["Wq", "Wk", "Wv", "Wqm", "Wkm", "Wvm", "Wout", "Wout_mol"]
_B_NAMES = ["bq", "bk", "bv", "bqm", "bkm", "bvm", "bout", "bout_mol"]


def _chunks(total, size):
    out = []
    o = 0
    while o < total:
        out.append((o, min(size, total - o)))
        o += size
    return out


def _build():
    nc = bacc.Bacc("TRN2", target_bir_lowering=False, debug=False,
                   num_devices=N_CORES)

    io = {}
    io["hidden_states"] = nc.dram_tensor("hidden_states", [P, D], F32,
                                         kind="ExternalInput")
    io["mol"] = nc.dram_tensor("mol", [M, D], F32, kind="ExternalInput")
    for w in _W_NAMES:
        io[w] = nc.dram_tensor(w, [D, D], F32, kind="ExternalInput")
    for b in _B_NAMES:
        io[b] = nc.dram_tensor(b, [D], F32, kind="ExternalInput")
    io["Wfc"] = nc.dram_tensor("Wfc", [S, P], F32, kind="ExternalInput")
    io["bfc"] = nc.dram_tensor("bfc", [P], F32, kind="ExternalInput")
    io["Wfc_mol"] = nc.dram_tensor("Wfc_mol", [S, M], F32, kind="ExternalInput")
    io["bfc_mol"] = nc.dram_tensor("bfc_mol", [M], F32, kind="ExternalInput")
    io["out_prot"] = nc.dram_tensor("out_prot", [P, D], F32,
                                    kind="ExternalOutput")
    io["out_mol"] = nc.dram_tensor("out_mol", [M, D], F32,
                                   kind="ExternalOutput")
    # DRAM scratch for the concatenated attention contexts (seq-major).
    cat_prot = nc.dram_tensor("cat_prot", [S, D], F32R, kind="Internal")
    cat_mol = nc.dram_tensor("cat_mol", [S, D], F32R, kind="Internal")

    with tile.TileContext(nc) as tc:
        _kernel(tc, io, cat_prot, cat_mol)
    nc.compile()
    return nc


def _kernel(tc, io, cat_prot, cat_mol):
    nc = tc.nc
    ap = {k: v.ap() for k, v in io.items()}
    catp = cat_prot.ap().rearrange("(t p) d -> p t d", p=128)
    catm = cat_mol.ap().rearrange("(t p) d -> p t d", p=128)

    ctx = contextlib.ExitStack()
    with ctx:
        const = ctx.enter_context(tc.tile_pool(name="const", bufs=1))
        psA = ctx.enter_context(tc.tile_pool(name="psA", bufs=2, space="PSUM"))
        psS = ctx.enter_context(tc.tile_pool(name="psS", bufs=1, space="PSUM"))
        psC = ctx.enter_context(tc.tile_pool(name="psC", bufs=2, space="PSUM"))

        ident = const.tile([128, 128], F32)
        make_identity(nc, ident[:])

        def bcast(name, n):
            t = const.tile([128, n], F32, name=f"bc_{name}")
            src = ap[name].rearrange("(o n) -> o n", o=1).to_broadcast([128, n])
            nc.sync.dma_start(t[:], src)
            return t

        def ppart(name):
            t = const.tile([128, DT], F32, name=f"pp_{name}")
            nc.sync.dma_start(t[:], ap[name].rearrange("(mo p) -> p mo", p=128))
            return t

        ones_c = const.tile([128, H], F32, name="ones_c")
        nc.vector.memset(ones_c[:], 1.0)

        # Activation pools, split by lifetime:
        #   actsQ: qT, qmT, kmT, vm_aug -- live until the pm stream is done
        #   actsK: kT, v_aug            -- dead after the mp stream
        actsQ = ctx.enter_context(tc.tile_pool(name="actsQ", bufs=1))
        qT = actsQ.tile([128, DT, P], F32R)
        qmT = actsQ.tile([128, DT, M], F32R)
        kmT = actsQ.tile([128, DT, M], F32R)
        vm_aug = actsQ.tile([128, MT, H * (DH + 1)], F32R)   # [128, 2, 780]

        actsK_cm = tc.tile_pool(name="actsK", bufs=1)
        actsK = actsK_cm.__enter__()
        kT = actsK.tile([128, DT, P], F32R)
        v_aug = actsK.tile([128, PT, H * (DH + 1)], F32R)    # [128, 8, 780]

        # ---- phase 1: transpose inputs to feature-major ----------------
        with tc.tile_pool(name="proj", bufs=1) as proj:
            hsT = proj.tile([128, DT, P], F32R)
            molT = proj.tile([128, DT, M], F32R)
            with tc.tile_pool(name="ld", bufs=3) as ld, \
                    nc.named_scope("transpose_in"):
                for (src, dstT, nt) in ((ap["hidden_states"], hsT, PT),
                                        (ap["mol"], molT, MT)):
                    for st in range(nt):
                        xs = ld.tile([128, D], F32, tag="xs")
                        nc.sync.dma_start(xs[:], src[st * 128:(st + 1) * 128, :])
                        for dt in range(DT):
                            pt = psA.tile([128, 512], F32, tag="psA")
                            nc.tensor.transpose(
                                pt[:, 0:128], xs[:, dt * 128:(dt + 1) * 128],
                                ident[:])
                            nc.vector.tensor_copy(
                                dstT[:, dt, st * 128:(st + 1) * 128],
                                pt[:, 0:128])

            # per-partition / broadcast bias tiles (needed from phase 2 on)
            bq_p, bk_p = ppart("bq"), ppart("bk")
            bqm_p, bkm_p = ppart("bqm"), ppart("bkm")
            bv_b = bcast("bv", D)
            bvm_b = bcast("bvm", D)
            bfc_b = bcast("bfc", P)
            bfcm_b = bcast("bfc_mol", M)
            bout_b = bcast("bout", D)
            boutm_b = bcast("bout_mol", D)

            # ---- phase 2: projections ---------------------------------
            with tc.tile_pool(name="wp", bufs=2) as wp, \
                    nc.named_scope("proj"):
                for (wname, bias_p, dstT, srcT, n_size) in (
                        ("Wq", bq_p, qT, hsT, P), ("Wk", bk_p, kT, hsT, P),
                        ("Wqm", bqm_p, qmT, molT, M),
                        ("Wkm", bkm_p, kmT, molT, M)):
                    w_sb = wp.tile([128, DT, D], F32R, tag="w", name=wname)
                    nc.gpsimd.dma_start(
                        w_sb[:], ap[wname].rearrange("(ko p) n -> p ko n", p=128))
                    for mo in range(DT):
                        for (n0, nsz) in _chunks(n_size, 512):
                            ps = psA.tile([128, 512], F32, tag="psA")
                            for ko in range(DT):
                                nc.tensor.matmul(
                                    ps[:, 0:nsz],
                                    w_sb[:, ko, mo * 128:(mo + 1) * 128],
                                    srcT[:, ko, n0:n0 + nsz],
                                    start=(ko == 0), stop=(ko == DT - 1))
                            nc.vector.tensor_scalar_add(
                                dstT[:, mo, n0:n0 + nsz], ps[:, 0:nsz],
                                bias_p[:, mo:mo + 1])

                for (wname, bias_b, dst, srcT, seq_t) in (
                        ("Wv", bv_b, v_aug, hsT, PT),
                        ("Wvm", bvm_b, vm_aug, molT, MT)):
                    w_sb = wp.tile([128, DT, D], F32R, tag="w", name=wname)
                    nc.gpsimd.dma_start(
                        w_sb[:], ap[wname].rearrange("(ko p) n -> p ko n", p=128))
                    for st in range(seq_t):
                        for (n0, nsz) in _chunks(D, 512):
                            ps = psA.tile([128, 512], F32, tag="psA")
                            for ko in range(DT):
                                nc.tensor.matmul(
                                    ps[:, 0:nsz],
                                    srcT[:, ko, st * 128:(st + 1) * 128],
                                    w_sb[:, ko, n0:n0 + nsz],
                                    start=(ko == 0), stop=(ko == DT - 1))
                            h0, hn = n0 // DH, nsz // DH
                            dst_v = dst[:, st].rearrange(
                                "p (h x) -> p h x", x=DH + 1)[:, h0:h0 + hn, 0:DH]
                            nc.vector.tensor_add(
                                dst_v,
                                ps[:, 0:nsz].rearrange("p (h x) -> p h x", x=DH),
                                bias_b[:, n0:n0 + nsz].rearrange(
                                    "p (h x) -> p h x", x=DH))
                    for st in range(seq_t):
                        nc.vector.tensor_copy(
                            dst[:, st].rearrange(
                                "p (h x) -> p h x", x=DH + 1)[:, :, DH],
                            ones_c[:])

        # ---- phase 3: attention ---------------------------------------
        def attention(qsrc, SQ, ksrc, SK, vaug, cat_dst, st_base, tagsfx):
            JT = SK // 128
            CH = 512 if SQ >= 512 else SQ
            G = 2
            with tc.tile_pool(name=f"at_{tagsfx}", bufs=2) as at, \
                    nc.named_scope(f"att_{tagsfx}"):
                for (i0, _) in _chunks(SQ, CH):
                    nst = CH // 128
                    stage = at.tile([128, nst, D], F32R, tag="stage")
                    for hp in range(H // 2):
                        h0, h1 = 2 * hp, 2 * hp + 1
                        ps_ct0 = psC.tile([128, CH], F32, tag="psC",
                                          name="ps_ct0")
                        ps_ct1 = psC.tile([128, CH], F32, tag="psC",
                                          name="ps_ct1")
                        for jg in range(0, JT, G):
                            gn = min(G, JT - jg)
                            ps_s0 = psS.tile([128, G, CH], F32, tag="psS0")
                            ps_s1 = psS.tile([128, G, CH], F32, tag="psS1")
                            # scores streak; h0/h1 on opposite row halves
                            # run concurrently in the PE array
                            for g in range(gn):
                                jt = jg + g
                                nc.tensor.matmul(
                                    ps_s0[:, g],
                                    ksrc[0:DH, hp, jt * 128:(jt + 1) * 128],
                                    qsrc[0:DH, hp, i0:i0 + CH],
                                    start=True, stop=True)
                                nc.tensor.matmul(
                                    ps_s1[:, g],
                                    ksrc[DH:128, hp, jt * 128:(jt + 1) * 128],
                                    qsrc[DH:128, hp, i0:i0 + CH],
                                    start=True, stop=True)
                            probs0 = at.tile([128, G, CH], F32R, tag="probs0")
                            probs1 = at.tile([128, G, CH], F32R, tag="probs1")
                            nc.scalar.activation(
                                probs0[:, 0:gn], ps_s0[:, 0:gn], AF.Exp,
                                scale=0.125)
                            nc.scalar.activation(
                                probs1[:, 0:gn], ps_s1[:, 0:gn], AF.Exp,
                                scale=0.125)
                            # ctx streak (K=128, M=65, N=CH)
                            for g in range(gn):
                                jt = jg + g
                                nc.tensor.matmul(
                                    ps_ct0[0:DH + 1, :],
                                    vaug[:, jt,
                                         h0 * (DH + 1):(h0 + 1) * (DH + 1)],
                                    probs0[:, g],
                                    start=(jt == 0), stop=(jt == JT - 1))
                                nc.tensor.matmul(
                                    ps_ct1[0:DH + 1, :],
                                    vaug[:, jt,
                                         h1 * (DH + 1):(h1 + 1) * (DH + 1)],
                                    probs1[:, g],
                                    start=(jt == 0), stop=(jt == JT - 1))
                        # tail: seq-major transpose + normalize, both heads
                        for (h, ps_ct, cj) in ((h0, ps_ct0, 0), (h1, ps_ct1, 1)):
                            ctx_sb = at.tile([DH + 1, CH], F32, tag=f"ctx{cj}")
                            nc.vector.tensor_copy(ctx_sb[:], ps_ct[0:DH + 1, :])
                            for ii in range(nst):
                                ps_t = psA.tile([128, 512], F32, tag="psA")
                                nc.tensor.transpose(
                                    ps_t[:, 0:DH + 1],
                                    ctx_sb[:, ii * 128:(ii + 1) * 128],
                                    ident[0:DH + 1, 0:DH + 1])
                                rec = at.tile([128, 1], F32, tag="rec", bufs=4)
                                nc.vector.reciprocal(rec[:], ps_t[:, DH:DH + 1])
                                nc.vector.tensor_scalar_mul(
                                    stage[:, ii, h * DH:(h + 1) * DH],
                                    ps_t[:, 0:DH], rec[:])
                    st = st_base + (i0 // 128)
                    nc.sync.dma_start(cat_dst[:, st:st + nst, :], stage[:])

        def fc_stage(cat_src, wname, bias_bc, dstT, NP, pool):
            wfc_sb = pool.tile([128, ST, NP], F32R, name=f"sb_{wname}")
            nc.gpsimd.dma_start(
                wfc_sb[:], ap[wname].rearrange("(ko p) n -> p ko n", p=128))
            cat_sb = pool.tile([128, ST, D], F32R, name=f"cat_{wname}")
            nc.sync.dma_start(cat_sb[:], cat_src[:])
            for mo in range(DT):
                for (n0, nsz) in _chunks(NP, 512):
                    ps = psA.tile([128, 512], F32, tag="psA")
                    for st in range(ST):
                        nc.tensor.matmul(
                            ps[:, 0:nsz],
                            cat_sb[:, st, mo * 128:(mo + 1) * 128],
                            wfc_sb[:, st, n0:n0 + nsz],
                            start=(st == 0), stop=(st == ST - 1))
                    nc.vector.tensor_add(
                        dstT[:, mo, n0:n0 + nsz], ps[:, 0:nsz],
                        bias_bc[:, n0:n0 + nsz])

        attention(qT, P, kT, P, v_aug, catp, 0, "pp")        # prot->prot
        attention(qmT, M, kT, P, v_aug, catp, PT, "mp")      # mol->prot kv
        actsK_cm.__exit__(None, None, None)

        # out_fcT / out_fc_molT live until the output projections
        acts2 = ctx.enter_context(tc.tile_pool(name="acts2", bufs=1))
        ofcT = acts2.tile([128, DT, P], F32R)
        ofcmT = acts2.tile([128, DT, M], F32R)

        # fc for the protein stream overlaps the mm/pm attention tail
        fcp_cm = tc.tile_pool(name="fcp", bufs=1)
        fcp = fcp_cm.__enter__()
        with nc.named_scope("fc_prot"):
            fc_stage(catp, "Wfc", bfc_b, ofcT, P, fcp)

        attention(qmT, M, kmT, M, vm_aug, catm, 0, "mm")     # mol->mol
        fcp_cm.__exit__(None, None, None)
        attention(qT, P, kmT, M, vm_aug, catm, MT, "pm")     # prot->mol kv

        with tc.tile_pool(name="fcm", bufs=1) as fcm, nc.named_scope("fc_mol"):
            fc_stage(catm, "Wfc_mol", bfcm_b, ofcmT, M, fcm)

        # ---- phase 5: output projections ------------------------------
        with tc.tile_pool(name="op", bufs=1) as op, nc.named_scope("outproj"):
            with tc.tile_pool(name="ost", bufs=3) as ost:
                for (srcT, wname, bias_bc, out_dram, n_tiles) in (
                        (ofcT, "Wout", bout_b, ap["out_prot"], PT),
                        (ofcmT, "Wout_mol", boutm_b, ap["out_mol"], MT)):
                    wo_sb = op.tile([128, DT, D], F32R, name=f"sb_{wname}")
                    nc.gpsimd.dma_start(
                        wo_sb[:], ap[wname].rearrange("(ko p) n -> p ko n", p=128))
                    for mo in range(n_tiles):
                        o_sb = ost.tile([128, D], F32, tag="osb")
                        for (n0, nsz) in _chunks(D, 512):
                            ps = psA.tile([128, 512], F32, tag="psA")
                            for kt in range(DT):
                                nc.tensor.matmul(
                                    ps[:, 0:nsz],
                                    srcT[:, kt, mo * 128:(mo + 1) * 128],
                                    wo_sb[:, kt, n0:n0 + nsz],
                                    start=(kt == 0), stop=(kt == DT - 1))
                            nc.vector.tensor_add(
                                o_sb[:, n0:n0 + nsz], ps[:, 0:nsz],
                                bias_bc[:, n0:n0 + nsz])
                        nc.sync.dma_start(
                            out_dram[mo * 128:(mo + 1) * 128, :], o_sb[:])


_NC_CACHE = None


def _get_program():
    global _NC_CACHE
    if _NC_CACHE is None:
        _NC_CACHE = _build()
    return _NC_CACHE


def kernel(**inputs):
    nc = _get_program()
    per_core_names = (["hidden_states", "mol"] + _W_NAMES + _B_NAMES
                      + ["Wfc", "bfc", "Wfc_mol", "bfc_mol"])
    in_maps = []
    for c in range(N_CORES):
        m = {}
        for name in per_core_names:
            arr = np.ascontiguousarray(np.asarray(inputs[name], dtype=np.float32))
            if name in ("hidden_states", "mol"):
                arr = arr[c]
            m[name] = arr
        in_maps.append(m)

    res = bass_utils.run_bass_kernel_spmd(nc, in_maps,
                                          core_ids=list(range(N_CORES)))
    global LAST_RESULTS
    LAST_RESULTS = res
    out_prot = np.stack([res.results[c]["out_prot"] for c in range(N_CORES)])
    out_mol = np.stack([res.results[c]["out_mol"] for c in range(N_CORES)])
    return out_prot, out_mol


LAST_RESULTS = None
